# revision 22
# baseline (speedup 1.0000x reference)
"""MoE ExpertRouter kernel for Trainium2 (8 NeuronCores, Bass/Tile).

Computation (matches the reference):
    router_logits = x @ W.T                       [8192, 64]
    noisy = router_logits + 0.1 * noise
    top2 values/indices of noisy per token        [8192, 2]
    expert_weights = softmax(top2 values)         [8192, 2]
    counts[e] = histogram of top2 indices
    load_balance_loss = 64 * sum(counts/8192 * mean softmax(router_logits))

Sharding: data-parallel over the token axis — each of the 8 cores gets
1024 tokens of x and noise; the small router weight W is replicated.
x is presented to each core in d-major ("x^T") layout, because the PE
systolic array contracts over the SBUF partition dimension: the host
sharding step hands core c the slice x[c*1024:(c+1)*1024].T so the
device can stream [128d x 512t] tiles directly into the matmul with no
on-chip transpose. All FLOPs (matmul, top-k, softmax, loss partials)
run on device; the host only slices/concats and combines the 8 per-core
64-vectors (counts, prob sums) into the scalar loss.

Matmul orientation: logits^T[64e, 512t] = sum_k wt_k[128d, 64e].T @
xk[128d, 512t] — W^T chunks are the stationary operand (64 columns,
cheap LDWEIGHTS) and x^T is the wide moving operand (N=512), which
minimizes PE instruction count for fp32 (walrus splits each fp32
matmul into hi/lo passes). The [64, 512] logits^T PSUM tile is then
flipped to token-major [128, 64] tiles with 4 PE transposes so the
top-k/softmax runs as cheap free-dim DVE reductions.

Measured (neuron-profile, core 0): ~92-97 us HW exec per invocation
(+-3 us run-to-run on identical NEFFs).
The kernel is jointly PE/DMA-bound: fp32 matmuls stream at 4 cycles per
moving column (walrus emits 2 HW passes per fp32 matmul), putting the
PE floor at ~55 us/core, while the 16.8 MB x-shard load is ~56 us of
SDMA time; x loads alternate between the SP and ACT HWDGE rings so the
SDMA engines round-robin two queues and hide HBM latency (~380 GB/s
observed). ~18 us is fixed Tile preamble + end-of-kernel barrier.
"""

import numpy as np

NCORES = 8
T_FULL = 8192
D = 4096
E = 64  # num experts
TOP_K = 2
NOISE_STD = 0.1

T = T_FULL // NCORES  # tokens per core
KCH = D // 128        # contraction chunks of 128
KPER = 4              # k-chunks per DMA
NBLK = 2              # token blocks per core (postproc pipelining)
BT = T // NBLK        # tokens per block
TPB = BT // 128       # 128-token tiles per block
NTILES = T // 128

# set by test harness for profiling runs
TRACE = False
TRACE_DIR = None

_cached = None


def _build():
    import concourse.bacc as bacc
    import concourse.mybir as mybir
    import concourse.tile as tile

    dt = mybir.dt
    f32 = dt.float32
    Alu = mybir.AluOpType
    Act = mybir.ActivationFunctionType
    AX = mybir.AxisListType

    nc = bacc.Bacc("TRN2", target_bir_lowering=False, debug=False,
                   num_devices=NCORES)

    xt = nc.dram_tensor("xt", [D, T], f32, kind="ExternalInput")
    wt = nc.dram_tensor("wt", [128, KCH * E], f32, kind="ExternalInput")
    noise = nc.dram_tensor("noise", [T, E], f32, kind="ExternalInput")
    logits_out = nc.dram_tensor("logits_out", [T, E], f32, kind="ExternalOutput")
    idx_out = nc.dram_tensor("idx_out", [T, TOP_K], dt.int32, kind="ExternalOutput")
    ew_out = nc.dram_tensor("ew_out", [T, TOP_K], f32, kind="ExternalOutput")
    stats_out = nc.dram_tensor("stats_out", [1, 2 * E], f32, kind="ExternalOutput")

    with tile.TileContext(nc) as tc:
        with (
            tc.tile_pool(name="const", bufs=1) as cpool,
            tc.tile_pool(name="xk", bufs=6) as xpool,
            tc.tile_pool(name="nz", bufs=2) as npool,
            tc.tile_pool(name="work", bufs=3) as wpool,
            tc.tile_pool(name="small", bufs=3) as spool,
            tc.tile_pool(name="lsb", bufs=2) as lpool,
            tc.tile_pool(name="lout", bufs=2) as opool,
            tc.tile_pool(name="ps", bufs=2, space="PSUM") as pspool,
            tc.tile_pool(name="tps", bufs=2, space="PSUM") as tpool,
            tc.tile_pool(name="statps", bufs=1, space="PSUM") as statpool,
        ):
            # --- constants ---
            # W^T chunk 0 loaded separately (tiny) so the first matmul
            # isn't gated on the full 1MB weight load
            wtile0 = cpool.tile([128, E], f32)
            nc.sync.dma_start(out=wtile0[:], in_=wt[:, 0:E])
            wtile = cpool.tile([128, KCH * E], f32)  # W^T as 32 [128,64] chunks
            nc.sync.dma_start(out=wtile[:], in_=wt[:, :])
            ones = cpool.tile([128, 1], f32)
            nc.vector.memset(ones[:], 1.0)
            # iotarev[p, j] = E - j  (so max over eq*iotarev picks LOWEST j)
            iotarev = cpool.tile([128, E], f32)
            nc.gpsimd.iota(iotarev[:], pattern=[[-1, E]], base=E,
                           channel_multiplier=0,
                           allow_small_or_imprecise_dtypes=True)
            # 64x64 identity for PE transposes
            ones64 = cpool.tile([64, E], f32)
            nc.vector.memset(ones64[:], 1.0)
            id64 = cpool.tile([64, E], f32)
            nc.gpsimd.affine_select(id64[:], ones64[:], pattern=[[-1, E]],
                                    compare_op=Alu.is_equal, fill=0.0,
                                    base=0, channel_multiplier=1)

            idx_stage = cpool.tile([128, TOP_K * NTILES], dt.int32)
            ew_stage = cpool.tile([128, TOP_K * NTILES], f32)
            countacc = cpool.tile([128, E], f32)
            nc.vector.memset(countacc[:], 0.0)
            probacc = cpool.tile([128, E], f32)
            nc.vector.memset(probacc[:], 0.0)

            for b in range(NBLK):
                # --- matmul: logitsT[e, t] = sum_k wt_k.T @ xk ---
                lps = pspool.tile([64, BT], f32, name=f"lps_{b}")
                for ci, kk in enumerate(range(0, KCH, KPER)):
                    kn = KPER
                    xk = xpool.tile([128, kn * BT], f32,
                                    name=f"xk_{b}_{kk}", tag="xk")
                    eng = nc.scalar if ci % 2 == 0 else nc.sync
                    eng.dma_start(
                        out=xk[:].rearrange("p (q t) -> p q t", q=kn),
                        in_=xt[kk * 128:(kk + kn) * 128,
                               b * BT:(b + 1) * BT].rearrange(
                                   "(q p) t -> p q t", p=128))
                    for q in range(kn):
                        k = kk + q
                        lhsT = (wtile0[:] if k == 0 else
                                wtile[:, k * E:(k + 1) * E])
                        nc.tensor.matmul(
                            lps[:], lhsT=lhsT,
                            rhs=xk[:, q * BT:(q + 1) * BT],
                            start=(k == 0), stop=(k == KCH - 1),
                        )

                ntile = npool.tile([128, TPB * E], f32, name=f"ntile_{b}",
                                   tag="ntile")
                nc.scalar.dma_start(
                    out=ntile[:].rearrange("p (i e) -> p i e", e=E),
                    in_=noise[b * BT:(b + 1) * BT, :].rearrange(
                        "(i p) e -> p i e", p=128),
                )
                # logitsT -> SBUF, then PE-transpose to token-major tiles
                lsbT = lpool.tile([64, BT], f32, name=f"lsbT_{b}", tag="lsbT")
                nc.scalar.copy(lsbT[:], lps[:])
                tps = tpool.tile([128, TPB * E], f32, name=f"tps_{b}", tag="tps")
                for i in range(TPB):
                    nc.tensor.transpose(tps[:, i * E:(i + 1) * E],
                                        lsbT[:, i * 128:(i + 1) * 128],
                                        id64[:])
                lstage = opool.tile([128, TPB * E], f32, name=f"lstage_{b}",
                                    tag="lstage")
                nc.scalar.copy(lstage[:], tps[:])

                # pre-scaled noise for this block (0.1 * noise)
                noise01 = npool.tile([128, TPB * E], f32, tag="noise01",
                                     name=f"noise01_{b}")
                nc.vector.tensor_scalar(noise01[:], ntile[:], NOISE_STD, None,
                                        Alu.mult)

                # --- per-tile post-processing ---
                for i in range(TPB):
                    tg = b * TPB + i  # global tile index
                    Lsb = lstage[:, i * E:(i + 1) * E]

                    # noisy = 0.1*noise + logits
                    noisy = wpool.tile([128, E], f32, tag="noisy")
                    nc.vector.tensor_tensor(noisy[:],
                                            noise01[:, i * E:(i + 1) * E],
                                            Lsb, Alu.add)
                    m1p = spool.tile([128, 1], f32, tag="m1p")  # +max1
                    nc.vector.tensor_reduce(m1p[:], noisy[:], axis=AX.X,
                                            op=Alu.max)
                    # top-1 argmax (lowest index wins on ties)
                    eq1 = wpool.tile([128, E], f32, tag="eq1")
                    nc.vector.tensor_scalar(eq1[:], noisy[:], m1p[:, 0:1], 0.0,
                                            Alu.subtract, Alu.is_equal)
                    sel1 = wpool.tile([128, E], f32, tag="sel1")
                    nc.vector.tensor_tensor(sel1[:], eq1[:], iotarev[:], Alu.mult)
                    rmx = spool.tile([128, TOP_K], f32, tag="rmx")
                    nc.vector.tensor_reduce(rmx[:, 0:1], sel1[:], axis=AX.X,
                                            op=Alu.max)

                    # top-2: mask out the argmax and repeat
                    masked = wpool.tile([128, E], f32, tag="masked")
                    nc.vector.scalar_tensor_tensor(
                        masked[:], eq1[:], -1e38, noisy[:], Alu.mult, Alu.add)
                    m2p = spool.tile([128, 1], f32, tag="m2p")  # +max2
                    nc.vector.tensor_reduce(m2p[:], masked[:], axis=AX.X,
                                            op=Alu.max)
                    eq2 = wpool.tile([128, E], f32, tag="eq2")
                    nc.vector.tensor_scalar(eq2[:], masked[:], m2p[:, 0:1], 0.0,
                                            Alu.subtract, Alu.is_equal)
                    sel2 = wpool.tile([128, E], f32, tag="sel2")
                    nc.vector.tensor_tensor(sel2[:], eq2[:], iotarev[:], Alu.mult)
                    nc.vector.tensor_reduce(rmx[:, 1:2], sel2[:], axis=AX.X,
                                            op=Alu.max)
                    # both indices in one conversion: idx = E - rmx
                    nc.vector.tensor_scalar(
                        idx_stage[:, tg * TOP_K:(tg + 1) * TOP_K],
                        rmx[:], -1.0, float(E), Alu.mult, Alu.add)

                    # expert weights via exp (avoids sigmoid table reload):
                    # u = exp(m2 - m1); w1 = 1/(1+u); w2 = 1 - w1
                    u = spool.tile([128, 1], f32, tag="u")
                    nc.scalar.activation(u[:], m1p[:], Act.Exp,
                                         bias=m2p[:, 0:1], scale=-1.0)
                    v = spool.tile([128, 1], f32, tag="v")
                    nc.scalar.add(v[:], u[:], 1.0)  # ACT: keep DVE chain short
                    w1 = ew_stage[:, tg * TOP_K:tg * TOP_K + 1]
                    nc.vector.reciprocal(w1, v[:])
                    nc.scalar.activation(
                        ew_stage[:, tg * TOP_K + 1:tg * TOP_K + 2],
                        w1, Act.Identity, bias=1.0, scale=-1.0)

                    # clean softmax -> prob tile (for mean router prob).
                    # logits are bounded (|L| < ~5) so no max-shift needed;
                    # softmax is shift-invariant.
                    et = wpool.tile([128, E], f32, tag="et")
                    ssum = spool.tile([128, 1], f32, tag="ssum")
                    nc.scalar.activation(et[:], Lsb, Act.Exp,
                                         bias=0.0, scale=1.0,
                                         accum_out=ssum[:])
                    rs = spool.tile([128, 1], f32, tag="rs")
                    nc.vector.reciprocal(rs[:], ssum[:])

                    # stats: per-expert accumulation across tokens (DVE)
                    nc.vector.tensor_tensor(countacc[:], eq1[:], countacc[:],
                                            Alu.add)
                    nc.vector.tensor_tensor(countacc[:], eq2[:], countacc[:],
                                            Alu.add)
                    nc.vector.scalar_tensor_tensor(
                        probacc[:], et[:], rs[:, 0:1], probacc[:],
                        Alu.mult, Alu.add)

                # block outputs -> DRAM
                nc.scalar.dma_start(
                    out=logits_out[b * BT:(b + 1) * BT, :].rearrange(
                        "(i p) e -> p i e", p=128),
                    in_=lstage[:].rearrange("p (i e) -> p i e", e=E),
                )
                nc.scalar.dma_start(
                    out=idx_out[b * BT:(b + 1) * BT, :].rearrange(
                        "(i p) j -> p i j", p=128),
                    in_=idx_stage[:, b * TPB * TOP_K:(b + 1) * TPB * TOP_K]
                        .rearrange("p (i j) -> p i j", j=TOP_K),
                )
                nc.scalar.dma_start(
                    out=ew_out[b * BT:(b + 1) * BT, :].rearrange(
                        "(i p) j -> p i j", p=128),
                    in_=ew_stage[:, b * TPB * TOP_K:(b + 1) * TPB * TOP_K]
                        .rearrange("p (i j) -> p i j", j=TOP_K),
                )

            # final small outputs: cross-partition sums via PE
            statp = statpool.tile([1, 2 * E], f32)
            nc.tensor.matmul(statp[0:1, 0:E], lhsT=ones[:], rhs=countacc[:],
                             start=True, stop=True)
            nc.tensor.matmul(statp[0:1, E:2 * E], lhsT=ones[:], rhs=probacc[:],
                             start=True, stop=True)
            stat_sb = cpool.tile([1, 2 * E], f32)
            nc.vector.tensor_copy(stat_sb[:], statp[:])
            nc.scalar.dma_start(out=stats_out[:, :], in_=stat_sb[:])

    nc.compile()
    return nc


def _get_program():
    global _cached
    if _cached is None:
        _cached = _build()
    return _cached


def kernel(x, W, noise):
    from concourse.bass_utils import run_bass_kernel_spmd

    x = np.asarray(x, dtype=np.float32)
    W = np.asarray(W, dtype=np.float32)
    noise = np.asarray(noise, dtype=np.float32)

    nc = _get_program()

    # prepack W^T into the exact SBUF tile layout: [p, k*E+e] = W[e, k*128+p]
    wt_host = np.ascontiguousarray(
        W.T.reshape(KCH, 128, E).transpose(1, 0, 2).reshape(128, KCH * E))
    in_maps = []
    for c in range(NCORES):
        sl = slice(c * T, (c + 1) * T)
        in_maps.append({
            "xt": np.ascontiguousarray(x[sl].T),  # [D, T] d-major token shard
            "wt": wt_host,
            "noise": np.ascontiguousarray(noise[sl]),
        })

    kwargs = {}
    if TRACE:
        kwargs = {"trace": True, "tmpdir": TRACE_DIR}
    try:
        res = run_bass_kernel_spmd(nc, in_maps, core_ids=list(range(NCORES)),
                                   **kwargs)
    except Exception:
        # one retry: a crashed prior process can leave the device needing
        # a reset; re-running is usually enough
        res = run_bass_kernel_spmd(nc, in_maps, core_ids=list(range(NCORES)),
                                   **kwargs)
    kernel.last_results = res.results
    if TRACE:
        kernel.last_exec_time_ns = res.exec_time_ns

    logits = np.concatenate([res.results[c]["logits_out"] for c in range(NCORES)])
    idx = np.concatenate([res.results[c]["idx_out"] for c in range(NCORES)])
    ew = np.concatenate([res.results[c]["ew_out"] for c in range(NCORES)])

    counts = np.zeros(E, np.float32)
    probsum = np.zeros(E, np.float32)
    for c in range(NCORES):
        st = res.results[c]["stats_out"][0]
        counts += st[:E]
        probsum += st[E:]
    # loss = E * sum(fraction_tokens * mean_router_prob)
    loss = np.float32(E * np.sum((counts / T_FULL) * (probsum / T_FULL)))

    return idx.astype(np.int32), ew, logits, loss


# revision 24
# speedup vs baseline: 1.1444x; 1.1444x over previous
"""MoE ExpertRouter kernel for Trainium2 (8 NeuronCores, Bass/Tile).

Computation (matches the reference):
    router_logits = x @ W.T                       [8192, 64]
    noisy = router_logits + 0.1 * noise
    top2 values/indices of noisy per token        [8192, 2]
    expert_weights = softmax(top2 values)         [8192, 2]
    counts[e] = histogram of top2 indices
    load_balance_loss = 64 * sum(counts/8192 * mean softmax(router_logits))

Sharding: data-parallel over the token axis — each of the 8 cores gets
1024 tokens of x and noise; the small router weight W is replicated.
x is presented to each core in d-major ("x^T") layout, because the PE
systolic array contracts over the SBUF partition dimension: the host
sharding step hands core c the slice x[c*1024:(c+1)*1024].T so the
device can stream [128d x 512t] tiles directly into the matmul with no
on-chip transpose. All FLOPs (matmul, top-k, softmax, loss partials)
run on device; the host only slices/concats and combines the 8 per-core
64-vectors (counts, prob sums) into the scalar loss.

Matmul orientation: logits^T[64e, 512t] = sum_k wt_k[128d, 64e].T @
xk[128d, 512t] — W^T chunks are the stationary operand (64 columns,
cheap LDWEIGHTS) and x^T is the wide moving operand (N=512), which
minimizes PE instruction count for fp32 (walrus splits each fp32
matmul into hi/lo passes). The [64, 512] logits^T PSUM tile is then
flipped to token-major [128, 64] tiles with 4 PE transposes so the
top-k/softmax runs as cheap free-dim DVE reductions.

Measured (neuron-profile, core 0): ~92-97 us HW exec per invocation
(+-3 us run-to-run on identical NEFFs).
The kernel is jointly PE/DMA-bound: fp32 matmuls stream at 4 cycles per
moving column (walrus emits 2 HW passes per fp32 matmul), putting the
PE floor at ~55 us/core, while the 16.8 MB x-shard load is ~56 us of
SDMA time; x loads alternate between the SP and ACT HWDGE rings so the
SDMA engines round-robin two queues and hide HBM latency (~380 GB/s
observed). ~18 us is fixed Tile preamble + end-of-kernel barrier.
"""

import numpy as np

NCORES = 8
T_FULL = 8192
D = 4096
E = 64  # num experts
TOP_K = 2
NOISE_STD = 0.1

T = T_FULL // NCORES  # tokens per core
KCH = D // 128        # contraction chunks of 128
KPER = 4              # k-chunks per DMA
NBLK = 2              # token blocks per core (postproc pipelining)
BT = T // NBLK        # tokens per block
TPB = BT // 128       # 128-token tiles per block
NTILES = T // 128

# set by test harness for profiling runs
TRACE = False
TRACE_DIR = None

_cached = None


def _build():
    import concourse.bacc as bacc
    import concourse.mybir as mybir
    import concourse.tile as tile

    dt = mybir.dt
    f32 = dt.float32
    Alu = mybir.AluOpType
    Act = mybir.ActivationFunctionType
    AX = mybir.AxisListType

    nc = bacc.Bacc("TRN2", target_bir_lowering=False, debug=False,
                   num_devices=NCORES)

    f32r = dt.float32r
    xt = nc.dram_tensor("xt", [D, T], f32r, kind="ExternalInput")
    wt = nc.dram_tensor("wt", [128, KCH * E], f32r, kind="ExternalInput")
    noise = nc.dram_tensor("noise", [T, E], f32, kind="ExternalInput")
    logits_out = nc.dram_tensor("logits_out", [T, E], f32, kind="ExternalOutput")
    idx_out = nc.dram_tensor("idx_out", [T, TOP_K], dt.int32, kind="ExternalOutput")
    ew_out = nc.dram_tensor("ew_out", [T, TOP_K], f32, kind="ExternalOutput")
    stats_out = nc.dram_tensor("stats_out", [1, 2 * E], f32, kind="ExternalOutput")

    with tile.TileContext(nc) as tc:
        with (
            tc.tile_pool(name="const", bufs=1) as cpool,
            tc.tile_pool(name="xk", bufs=6) as xpool,
            tc.tile_pool(name="nz", bufs=2) as npool,
            tc.tile_pool(name="work", bufs=3) as wpool,
            tc.tile_pool(name="small", bufs=3) as spool,
            tc.tile_pool(name="lsb", bufs=2) as lpool,
            tc.tile_pool(name="lout", bufs=2) as opool,
            tc.tile_pool(name="ps", bufs=2, space="PSUM") as pspool,
            tc.tile_pool(name="tps", bufs=2, space="PSUM") as tpool,
            tc.tile_pool(name="statps", bufs=1, space="PSUM") as statpool,
        ):
            # --- constants ---
            # W^T chunk 0 loaded separately (tiny) so the first matmul
            # isn't gated on the full 1MB weight load
            wtile0 = cpool.tile([128, E], f32r)
            nc.sync.dma_start(out=wtile0[:], in_=wt[:, 0:E])
            wtile = cpool.tile([128, KCH * E], f32r)  # W^T as 32 [128,64] chunks
            nc.sync.dma_start(out=wtile[:], in_=wt[:, :])
            ones = cpool.tile([128, 1], f32)
            nc.vector.memset(ones[:], 1.0)
            # iotarev[p, j] = E - j  (so max over eq*iotarev picks LOWEST j)
            iotarev = cpool.tile([128, E], f32)
            nc.gpsimd.iota(iotarev[:], pattern=[[-1, E]], base=E,
                           channel_multiplier=0,
                           allow_small_or_imprecise_dtypes=True)
            # 64x64 identity for PE transposes
            ones64 = cpool.tile([64, E], f32)
            nc.vector.memset(ones64[:], 1.0)
            id64 = cpool.tile([64, E], f32)
            nc.gpsimd.affine_select(id64[:], ones64[:], pattern=[[-1, E]],
                                    compare_op=Alu.is_equal, fill=0.0,
                                    base=0, channel_multiplier=1)

            idx_stage = cpool.tile([128, TOP_K * NTILES], dt.int32)
            ew_stage = cpool.tile([128, TOP_K * NTILES], f32)
            countacc = cpool.tile([128, E], f32)
            nc.vector.memset(countacc[:], 0.0)
            probacc = cpool.tile([128, E], f32)
            nc.vector.memset(probacc[:], 0.0)

            for b in range(NBLK):
                # --- matmul: logitsT[e, t] = sum_k wt_k.T @ xk ---
                lps = pspool.tile([64, BT], f32, name=f"lps_{b}")
                for ci, kk in enumerate(range(0, KCH, KPER)):
                    kn = KPER
                    xk = xpool.tile([128, kn * BT], f32r,
                                    name=f"xk_{b}_{kk}", tag="xk")
                    eng = nc.scalar if ci % 2 == 0 else nc.sync
                    eng.dma_start(
                        out=xk[:].rearrange("p (q t) -> p q t", q=kn),
                        in_=xt[kk * 128:(kk + kn) * 128,
                               b * BT:(b + 1) * BT].rearrange(
                                   "(q p) t -> p q t", p=128))
                    for q in range(kn):
                        k = kk + q
                        lhsT = (wtile0[:] if k == 0 else
                                wtile[:, k * E:(k + 1) * E])
                        # float32r: single-pass fp32 matmul (4x the fp32
                        # rate at N>=256); precision verified against the
                        # fp32 reference in test.py
                        nc.tensor.matmul(
                            lps[:], lhsT=lhsT,
                            rhs=xk[:, q * BT:(q + 1) * BT],
                            start=(k == 0), stop=(k == KCH - 1),
                        )

                ntile = npool.tile([128, TPB * E], f32, name=f"ntile_{b}",
                                   tag="ntile")
                nc.scalar.dma_start(
                    out=ntile[:].rearrange("p (i e) -> p i e", e=E),
                    in_=noise[b * BT:(b + 1) * BT, :].rearrange(
                        "(i p) e -> p i e", p=128),
                )
                # logitsT -> SBUF, then PE-transpose to token-major tiles
                lsbT = lpool.tile([64, BT], f32, name=f"lsbT_{b}", tag="lsbT")
                nc.scalar.copy(lsbT[:], lps[:])
                tps = tpool.tile([128, TPB * E], f32, name=f"tps_{b}", tag="tps")
                for i in range(TPB):
                    nc.tensor.transpose(tps[:, i * E:(i + 1) * E],
                                        lsbT[:, i * 128:(i + 1) * 128],
                                        id64[:])
                lstage = opool.tile([128, TPB * E], f32, name=f"lstage_{b}",
                                    tag="lstage")
                nc.scalar.copy(lstage[:], tps[:])

                # pre-scaled noise for this block (0.1 * noise)
                noise01 = npool.tile([128, TPB * E], f32, tag="noise01",
                                     name=f"noise01_{b}")
                nc.vector.tensor_scalar(noise01[:], ntile[:], NOISE_STD, None,
                                        Alu.mult)

                # --- per-tile post-processing ---
                for i in range(TPB):
                    tg = b * TPB + i  # global tile index
                    Lsb = lstage[:, i * E:(i + 1) * E]

                    # noisy = 0.1*noise + logits
                    noisy = wpool.tile([128, E], f32, tag="noisy")
                    nc.vector.tensor_tensor(noisy[:],
                                            noise01[:, i * E:(i + 1) * E],
                                            Lsb, Alu.add)
                    m1p = spool.tile([128, 1], f32, tag="m1p")  # +max1
                    nc.vector.tensor_reduce(m1p[:], noisy[:], axis=AX.X,
                                            op=Alu.max)
                    # top-1 argmax (lowest index wins on ties)
                    eq1 = wpool.tile([128, E], f32, tag="eq1")
                    nc.vector.tensor_scalar(eq1[:], noisy[:], m1p[:, 0:1], 0.0,
                                            Alu.subtract, Alu.is_equal)
                    sel1 = wpool.tile([128, E], f32, tag="sel1")
                    nc.vector.tensor_tensor(sel1[:], eq1[:], iotarev[:], Alu.mult)
                    rmx = spool.tile([128, TOP_K], f32, tag="rmx")
                    nc.vector.tensor_reduce(rmx[:, 0:1], sel1[:], axis=AX.X,
                                            op=Alu.max)

                    # top-2: mask out the argmax and repeat
                    masked = wpool.tile([128, E], f32, tag="masked")
                    nc.vector.scalar_tensor_tensor(
                        masked[:], eq1[:], -1e38, noisy[:], Alu.mult, Alu.add)
                    m2p = spool.tile([128, 1], f32, tag="m2p")  # +max2
                    nc.vector.tensor_reduce(m2p[:], masked[:], axis=AX.X,
                                            op=Alu.max)
                    eq2 = wpool.tile([128, E], f32, tag="eq2")
                    nc.vector.tensor_scalar(eq2[:], masked[:], m2p[:, 0:1], 0.0,
                                            Alu.subtract, Alu.is_equal)
                    sel2 = wpool.tile([128, E], f32, tag="sel2")
                    nc.vector.tensor_tensor(sel2[:], eq2[:], iotarev[:], Alu.mult)
                    nc.vector.tensor_reduce(rmx[:, 1:2], sel2[:], axis=AX.X,
                                            op=Alu.max)
                    # both indices in one conversion: idx = E - rmx
                    nc.vector.tensor_scalar(
                        idx_stage[:, tg * TOP_K:(tg + 1) * TOP_K],
                        rmx[:], -1.0, float(E), Alu.mult, Alu.add)

                    # expert weights via exp (avoids sigmoid table reload):
                    # u = exp(m2 - m1); w1 = 1/(1+u); w2 = 1 - w1
                    u = spool.tile([128, 1], f32, tag="u")
                    nc.scalar.activation(u[:], m1p[:], Act.Exp,
                                         bias=m2p[:, 0:1], scale=-1.0)
                    v = spool.tile([128, 1], f32, tag="v")
                    nc.scalar.add(v[:], u[:], 1.0)  # ACT: keep DVE chain short
                    w1 = ew_stage[:, tg * TOP_K:tg * TOP_K + 1]
                    nc.vector.reciprocal(w1, v[:])
                    nc.scalar.activation(
                        ew_stage[:, tg * TOP_K + 1:tg * TOP_K + 2],
                        w1, Act.Identity, bias=1.0, scale=-1.0)

                    # clean softmax -> prob tile (for mean router prob).
                    # logits are bounded (|L| < ~5) so no max-shift needed;
                    # softmax is shift-invariant.
                    et = wpool.tile([128, E], f32, tag="et")
                    ssum = spool.tile([128, 1], f32, tag="ssum")
                    nc.scalar.activation(et[:], Lsb, Act.Exp,
                                         bias=0.0, scale=1.0,
                                         accum_out=ssum[:])
                    rs = spool.tile([128, 1], f32, tag="rs")
                    nc.vector.reciprocal(rs[:], ssum[:])

                    # stats: per-expert accumulation across tokens (DVE)
                    nc.vector.tensor_tensor(countacc[:], eq1[:], countacc[:],
                                            Alu.add)
                    nc.vector.tensor_tensor(countacc[:], eq2[:], countacc[:],
                                            Alu.add)
                    nc.vector.scalar_tensor_tensor(
                        probacc[:], et[:], rs[:, 0:1], probacc[:],
                        Alu.mult, Alu.add)

                # block outputs -> DRAM
                nc.scalar.dma_start(
                    out=logits_out[b * BT:(b + 1) * BT, :].rearrange(
                        "(i p) e -> p i e", p=128),
                    in_=lstage[:].rearrange("p (i e) -> p i e", e=E),
                )
                nc.scalar.dma_start(
                    out=idx_out[b * BT:(b + 1) * BT, :].rearrange(
                        "(i p) j -> p i j", p=128),
                    in_=idx_stage[:, b * TPB * TOP_K:(b + 1) * TPB * TOP_K]
                        .rearrange("p (i j) -> p i j", j=TOP_K),
                )
                nc.scalar.dma_start(
                    out=ew_out[b * BT:(b + 1) * BT, :].rearrange(
                        "(i p) j -> p i j", p=128),
                    in_=ew_stage[:, b * TPB * TOP_K:(b + 1) * TPB * TOP_K]
                        .rearrange("p (i j) -> p i j", j=TOP_K),
                )

            # final small outputs: cross-partition sums via PE
            statp = statpool.tile([1, 2 * E], f32)
            nc.tensor.matmul(statp[0:1, 0:E], lhsT=ones[:], rhs=countacc[:],
                             start=True, stop=True)
            nc.tensor.matmul(statp[0:1, E:2 * E], lhsT=ones[:], rhs=probacc[:],
                             start=True, stop=True)
            stat_sb = cpool.tile([1, 2 * E], f32)
            nc.vector.tensor_copy(stat_sb[:], statp[:])
            nc.scalar.dma_start(out=stats_out[:, :], in_=stat_sb[:])

    nc.compile()
    return nc


def _get_program():
    global _cached
    if _cached is None:
        _cached = _build()
    return _cached


def kernel(x, W, noise):
    from concourse.bass_utils import run_bass_kernel_spmd

    x = np.asarray(x, dtype=np.float32)
    W = np.asarray(W, dtype=np.float32)
    noise = np.asarray(noise, dtype=np.float32)

    nc = _get_program()

    # prepack W^T into the exact SBUF tile layout: [p, k*E+e] = W[e, k*128+p]
    wt_host = np.ascontiguousarray(
        W.T.reshape(KCH, 128, E).transpose(1, 0, 2).reshape(128, KCH * E))
    in_maps = []
    for c in range(NCORES):
        sl = slice(c * T, (c + 1) * T)
        in_maps.append({
            "xt": np.ascontiguousarray(x[sl].T),  # [D, T] d-major token shard
            "wt": wt_host,
            "noise": np.ascontiguousarray(noise[sl]),
        })

    kwargs = {}
    if TRACE:
        kwargs = {"trace": True, "tmpdir": TRACE_DIR}
    try:
        res = run_bass_kernel_spmd(nc, in_maps, core_ids=list(range(NCORES)),
                                   **kwargs)
    except Exception:
        # one retry: a crashed prior process can leave the device needing
        # a reset; re-running is usually enough
        res = run_bass_kernel_spmd(nc, in_maps, core_ids=list(range(NCORES)),
                                   **kwargs)
    kernel.last_results = res.results
    if TRACE:
        kernel.last_exec_time_ns = res.exec_time_ns

    logits = np.concatenate([res.results[c]["logits_out"] for c in range(NCORES)])
    idx = np.concatenate([res.results[c]["idx_out"] for c in range(NCORES)])
    ew = np.concatenate([res.results[c]["ew_out"] for c in range(NCORES)])

    counts = np.zeros(E, np.float32)
    probsum = np.zeros(E, np.float32)
    for c in range(NCORES):
        st = res.results[c]["stats_out"][0]
        counts += st[:E]
        probsum += st[E:]
    # loss = E * sum(fraction_tokens * mean_router_prob)
    loss = np.float32(E * np.sum((counts / T_FULL) * (probsum / T_FULL)))

    return idx.astype(np.int32), ew, logits, loss
